# revision 7
# baseline (speedup 1.0000x reference)
"""Trainium2 Bass kernel for nn_Decoder (degenerate LSTM decoder).

Math (see reference):
  gates = x @ W_ih^T + (b_ih + b_hh)      [B, T, 4D], gate order i, f, g, o
  c = sigmoid(i) * tanh(g)                (f unused: c0 = 0)
  h = sigmoid(o) * tanh(c)                [B, T, D]
  out = softmax((h.reshape(B, T*D) @ W_out^T + b_out).reshape(B, 4, 10), axis=2)

Strategy: pure data parallel over 8 cores (batch 2048 -> 256/core).
Per core, batch stays on SBUF partitions everywhere:
  - host pre-transposes x to [d+1, btile, t, b] fp16 (ones row folds the bias
    into the K=91 contraction), so mm1 is lhsT=x_t^T [91,128], rhs=W [91,270]
    -> gates [128b, 270] in PSUM (col order i, o, g).
  - ACT applies sigmoid to [0:180] (i,o) and tanh to [180:270] (g) straight
    from PSUM in 3-timestep batches; DVE does the two fp16 multiplies.
  - h [128, 21632] fp16 is transposed in 128-wide td-chunks by the DMA xbar;
    mm2 accumulates logitsT [40, 128] over 169 chunks in PSUM.
  - logitsT + b_out, PE-transpose to [128, 40], exp with per-group accum sums,
    reciprocal, scale -> softmax; store fp32.
"""

import numpy as np
from contextlib import ExitStack

import concourse.bass as bass
import concourse.bacc as bacc
import concourse.tile as tile
from concourse import mybir
from concourse.bass_utils import run_bass_kernel_spmd

F16 = mybir.dt.float16
F32 = mybir.dt.float32
AF = mybir.ActivationFunctionType

B, T, D = 2048, 240, 90
NCLS, NGRP, GRP = 40, 4, 10
NCORES = 8
BC = B // NCORES            # 256 batch rows per core
NBT = 2                     # 128-row btiles per core
K1 = D + 1                  # 91 = d + ones row (bias)
NG = 3 * D                  # 270 gate cols: i(90) o(90) g(90)
TD = T * D                  # 21600
NCH = (TD + 127) // 128     # 169
TDP = NCH * 128             # 21632
TBLK = 12                   # t-block for DMA/elementwise staging
GSLOTS = 6                  # gates PSUM rotation depth (6 banks)
ABATCH = 3                  # timesteps per ACT instruction over gates PSUM

_CACHE: dict = {}


def _build_nc():
    nc = bacc.Bacc("TRN2", target_bir_lowering=False, debug=False)

    xt_d = nc.dram_tensor("xt", [K1, NBT, T, 128], F16, kind="ExternalInput")
    wmov_d = nc.dram_tensor("wmov", [K1, NG], F16, kind="ExternalInput")
    wout_d = nc.dram_tensor("woutT", [128, NCH, NCLS], F16, kind="ExternalInput")
    bout_d = nc.dram_tensor("bout", [NCLS, 128], F32, kind="ExternalInput")
    id40_d = nc.dram_tensor("id40", [NCLS, NCLS], F32, kind="ExternalInput")
    out_d = nc.dram_tensor("out", [BC, NCLS], F32, kind="ExternalOutput")

    with ExitStack() as ctx:
        tc = ctx.enter_context(tile.TileContext(nc))
        consts = ctx.enter_context(tc.tile_pool(name="consts", bufs=1))
        xt_pool = ctx.enter_context(tc.tile_pool(name="xt", bufs=3))
        act_pool = ctx.enter_context(tc.tile_pool(name="acts", bufs=2))
        h_pool = ctx.enter_context(tc.tile_pool(name="h", bufs=2))
        ht_pool = ctx.enter_context(tc.tile_pool(name="ht", bufs=4))
        fin_pool = ctx.enter_context(tc.tile_pool(name="fin", bufs=2))
        pg_pool = ctx.enter_context(tc.tile_pool(name="pg", bufs=1, space="PSUM"))
        pl_pool = ctx.enter_context(tc.tile_pool(name="pl", bufs=2, space="PSUM"))

        wmov = consts.tile([K1, NG], F16)
        nc.gpsimd.dma_start(out=wmov[:], in_=wmov_d[:])
        wout = consts.tile([128, NCH, NCLS], F16)
        nc.gpsimd.dma_start(out=wout[:], in_=wout_d[:])
        bout = consts.tile([NCLS, 128], F32)
        nc.gpsimd.dma_start(out=bout[:], in_=bout_d[:])
        id40 = consts.tile([NCLS, NCLS], F32)
        nc.gpsimd.dma_start(out=id40[:], in_=id40_d[:])

        logTs = []
        for bt in range(NBT):
            h = h_pool.tile([128, TDP], F16)
            # zero the tail so the padded td-chunk contributes nothing
            nc.vector.memset(h[:, TD:TDP], 0.0)
            gates = pg_pool.tile([128, GSLOTS, 512], F32)
            logT = pl_pool.tile([NCLS, 128], F32, tag="lg")
            ck = 0
            for tb in range(T // TBLK):
                xt = xt_pool.tile([K1, TBLK, 128], F16)
                nc.gpsimd.dma_start(
                    out=xt[:], in_=xt_d[:, bt, tb * TBLK:(tb + 1) * TBLK, :]
                )
                sio = act_pool.tile([128, TBLK, 2 * D], F16)
                tg = act_pool.tile([128, TBLK, D], F16)
                cc = act_pool.tile([128, TBLK, D], F16)
                tcc = act_pool.tile([128, TBLK, D], F16)
                for j in range(TBLK):
                    t = tb * TBLK + j
                    s = t % GSLOTS
                    nc.tensor.matmul(
                        gates[:, s, 0:NG],
                        xt[:, j, :],
                        wmov[:],
                        start=True,
                        stop=True,
                        skip_group_check=True,
                    )
                    if s % ABATCH == ABATCH - 1:
                        j0, s0 = j - (ABATCH - 1), s - (ABATCH - 1)
                        nc.scalar.activation(
                            sio[:, j0:j + 1, :],
                            gates[:, s0:s + 1, 0:2 * D],
                            AF.Sigmoid,
                        )
                        nc.scalar.activation(
                            tg[:, j0:j + 1, :],
                            gates[:, s0:s + 1, 2 * D:NG],
                            AF.Tanh,
                        )
                # c = sigmoid(i) * tanh(g); h = sigmoid(o) * tanh(c)
                nc.vector.tensor_mul(cc[:], sio[:, :, 0:D], tg[:])
                nc.scalar.activation(tcc[:], cc[:], AF.Tanh)
                hv = h[:, tb * TBLK * D:(tb + 1) * TBLK * D].rearrange(
                    "p (t d) -> p t d", d=D
                )
                nc.vector.tensor_mul(hv, sio[:, :, D:2 * D], tcc[:])
                # drain completed 128-wide td-chunks into mm2
                while (ck + 1) * 128 <= (tb + 1) * TBLK * D:
                    ht = ht_pool.tile([128, 128], F16)
                    nc.sync.dma_start(
                        out=ht[:], in_=h[:, ck * 128:(ck + 1) * 128], transpose=True
                    )
                    nc.tensor.matmul(
                        logT[:],
                        wout[:, ck, :],
                        ht[:],
                        start=(ck == 0),
                        stop=(ck == NCH - 1),
                        skip_group_check=True,
                    )
                    ck += 1
            while ck < NCH:  # padded tail chunk
                ht = ht_pool.tile([128, 128], F16)
                nc.sync.dma_start(
                    out=ht[:], in_=h[:, ck * 128:(ck + 1) * 128], transpose=True
                )
                nc.tensor.matmul(
                    logT[:],
                    wout[:, ck, :],
                    ht[:],
                    start=(ck == 0),
                    stop=(ck == NCH - 1),
                    skip_group_check=True,
                )
                ck += 1
            logTs.append(logT)

        # softmax for both btiles at the end (one exp table load)
        for bt in range(NBT):
            logT = logTs[bt]
            lsb = fin_pool.tile([NCLS, 128], F32)
            nc.vector.tensor_add(lsb[:], logT[:], bout[:])
            smax = pl_pool.tile([128, NCLS], F32, tag="lg")
            nc.tensor.transpose(smax[:], lsb[:], id40[:])
            esb = fin_pool.tile([128, NCLS], F32)
            sums = fin_pool.tile([128, NGRP], F32)
            for g in range(NGRP):
                nc.scalar.activation(
                    esb[:, g * GRP:(g + 1) * GRP],
                    smax[:, g * GRP:(g + 1) * GRP],
                    AF.Exp,
                    accum_out=sums[:, g:g + 1],
                )
            rcp = fin_pool.tile([128, NGRP], F32)
            nc.vector.reciprocal(rcp[:], sums[:])
            ob = fin_pool.tile([128, NCLS], F32)
            for g in range(NGRP):
                nc.scalar.activation(
                    ob[:, g * GRP:(g + 1) * GRP],
                    esb[:, g * GRP:(g + 1) * GRP],
                    AF.Copy,
                    scale=rcp[:, g:g + 1],
                )
            nc.sync.dma_start(out=out_d[bt * 128:(bt + 1) * 128, :], in_=ob[:])

    nc.compile()
    return nc


def get_nc():
    if "nc" not in _CACHE:
        _CACHE["nc"] = _build_nc()
    return _CACHE["nc"]


def make_in_maps(x, W_ih, W_hh, b_ih, b_hh, W_out, b_out):
    f16 = mybir.dt.np(F16)
    bias = (np.asarray(b_ih, np.float32) + np.asarray(b_hh, np.float32))

    # moving operand [91, 270]: cols [i, o, g]; row 90 carries the bias
    wmov = np.zeros((K1, NG), np.float32)
    wmov[0:D, 0:D] = np.asarray(W_ih)[0:D].T            # i
    wmov[0:D, D:2 * D] = np.asarray(W_ih)[3 * D:4 * D].T  # o
    wmov[0:D, 2 * D:NG] = np.asarray(W_ih)[2 * D:3 * D].T  # g
    wmov[D, 0:D] = bias[0:D]
    wmov[D, D:2 * D] = bias[3 * D:4 * D]
    wmov[D, 2 * D:NG] = bias[2 * D:3 * D]
    wmov = wmov.astype(f16)

    wt = np.zeros((TDP, NCLS), np.float32)
    wt[0:TD] = np.asarray(W_out, np.float32).T
    wout = np.ascontiguousarray(
        wt.reshape(NCH, 128, NCLS).transpose(1, 0, 2)
    ).astype(f16)

    boutr = np.ascontiguousarray(
        np.broadcast_to(np.asarray(b_out, np.float32)[:, None], (NCLS, 128))
    )
    id40 = np.eye(NCLS, dtype=np.float32)

    # x [2048, 240, 90] -> per core [91, btile, t, b] fp16 with ones row
    xs = np.asarray(x, np.float32).reshape(NCORES, NBT, 128, T, D)
    xt_all = np.empty((NCORES, K1, NBT, T, 128), f16)
    xt_all[:, 0:D] = xs.transpose(0, 4, 1, 3, 2).astype(f16)
    xt_all[:, D] = np.array(1.0, f16)

    return [
        {
            "xt": np.ascontiguousarray(xt_all[c]),
            "wmov": wmov,
            "woutT": wout,
            "bout": boutr,
            "id40": id40,
        }
        for c in range(NCORES)
    ]


def kernel(x, W_ih, W_hh, b_ih, b_hh, W_out, b_out, trace=False, **run_kwargs):
    nc = get_nc()
    in_maps = make_in_maps(x, W_ih, W_hh, b_ih, b_hh, W_out, b_out)
    res = run_bass_kernel_spmd(
        nc, in_maps, list(range(NCORES)), trace=trace, **run_kwargs
    )
    out = np.concatenate([res.results[c]["out"] for c in range(NCORES)], axis=0)
    out = out.reshape(B, NGRP, GRP).astype(np.float32)
    if trace:
        kernel.last_result = res
    return out


# revision 11
# speedup vs baseline: 1.9274x; 1.9274x over previous
"""Trainium2 Bass kernel for nn_Decoder (degenerate LSTM decoder).

Math (see reference):
  gates = x @ W_ih^T + (b_ih + b_hh)      [B, T, 4D], gate order i, f, g, o
  c = sigmoid(i) * tanh(g)                (f unused: c0 = 0)
  h = sigmoid(o) * tanh(c)                [B, T, D]
  out = softmax((h.reshape(B, T*D) @ W_out^T + b_out).reshape(B, 4, 10), axis=2)

Strategy: pure data parallel over 8 cores (batch 2048 -> 256/core).
Per core, batch stays on SBUF partitions everywhere:
  - host pre-transposes x to [d+1, btile, t, b] fp16 (ones row folds the bias
    into the K=91 contraction), so mm1 is lhsT=x_t^T [91,128], rhs=W [91,270]
    -> gates [128b, 270] in PSUM (col order i, o, g).
  - ACT applies sigmoid to [0:180] (i,o) and tanh to [180:270] (g) straight
    from PSUM in 3-timestep batches; DVE does the two fp16 multiplies.
  - h [128, 21632] fp16 is transposed in 128-wide td-chunks by the DMA xbar;
    mm2 accumulates logitsT [40, 128] over 169 chunks in PSUM.
  - logitsT + b_out, PE-transpose to [128, 40], exp with per-group accum sums,
    reciprocal, scale -> softmax; store fp32.
"""

import numpy as np
from contextlib import ExitStack

import concourse.bass as bass
import concourse.bacc as bacc
import concourse.tile as tile
from concourse import mybir
from concourse.bass_utils import run_bass_kernel_spmd

F16 = mybir.dt.float16
F32 = mybir.dt.float32
AF = mybir.ActivationFunctionType

B, T, D = 2048, 240, 90
NCLS, NGRP, GRP = 40, 4, 10
NCORES = 8
BC = B // NCORES            # 256 batch rows per core
NBT = 2                     # 128-row btiles per core
K1 = D + 1                  # 91 = d + ones row (bias)
NG = 3 * D                  # 270 gate cols: i(90) o(90) g(90)
TD = T * D                  # 21600
NCH = (TD + 127) // 128     # 169
TDP = NCH * 128             # 21632
TBLK = 12                   # t-block for DMA/elementwise staging
GSLOTS = 6                  # gates PSUM rotation depth (6 banks)
ABATCH = 3                  # timesteps per ACT instruction over gates PSUM
TGRP = 13                   # td-chunks per batched DMA transpose (13*13=169)

_CACHE: dict = {}


def _build_nc():
    nc = bacc.Bacc("TRN2", target_bir_lowering=False, debug=False)

    xt_d = nc.dram_tensor("xt", [K1, NBT, T, 128], F16, kind="ExternalInput")
    wmov_d = nc.dram_tensor("wmov", [K1, NG], F16, kind="ExternalInput")
    wout_d = nc.dram_tensor("woutT", [128, NCH, NCLS], F16, kind="ExternalInput")
    bout_d = nc.dram_tensor("bout", [NCLS, 128], F32, kind="ExternalInput")
    id40_d = nc.dram_tensor("id40", [NCLS, NCLS], F32, kind="ExternalInput")
    out_d = nc.dram_tensor("out", [BC, NCLS], F32, kind="ExternalOutput")

    with ExitStack() as ctx:
        tc = ctx.enter_context(tile.TileContext(nc))
        consts = ctx.enter_context(tc.tile_pool(name="consts", bufs=1))
        xt_pool = ctx.enter_context(tc.tile_pool(name="xt", bufs=4))
        act_pool = ctx.enter_context(tc.tile_pool(name="acts", bufs=4))
        h_pool = ctx.enter_context(tc.tile_pool(name="h", bufs=2))
        ht_pool = ctx.enter_context(tc.tile_pool(name="ht", bufs=3))
        fin_pool = ctx.enter_context(tc.tile_pool(name="fin", bufs=2))
        pg_pool = ctx.enter_context(tc.tile_pool(name="pg", bufs=1, space="PSUM"))
        pl_pool = ctx.enter_context(tc.tile_pool(name="pl", bufs=2, space="PSUM"))

        wmov = consts.tile([K1, NG], F16)
        nc.gpsimd.dma_start(out=wmov[:], in_=wmov_d[:])
        wout = consts.tile([128, NCH, NCLS], F16)
        nc.gpsimd.dma_start(out=wout[:], in_=wout_d[:])
        bout = consts.tile([NCLS, 128], F32)
        nc.gpsimd.dma_start(out=bout[:], in_=bout_d[:])
        id40 = consts.tile([NCLS, NCLS], F32)
        nc.gpsimd.dma_start(out=id40[:], in_=id40_d[:])

        logTs = []
        for bt in range(NBT):
            h = h_pool.tile([128, TDP], F16)
            # zero the tail so the padded td-chunk contributes nothing
            nc.vector.memset(h[:, TD:TDP], 0.0)
            gates = pg_pool.tile([128, GSLOTS, 512], F32)
            logT = pl_pool.tile([NCLS, 128], F32, tag="lg")

            def mm2_group(g):
                # batched xbar transpose of TGRP td-chunks, then accumulate
                ht = ht_pool.tile([128, TGRP, 128], F16)
                c0 = g * TGRP
                nc.sync.dma_start(
                    out=ht[:],
                    in_=h[:, c0 * 128:(c0 + TGRP) * 128],
                    transpose=True,
                )
                for i in range(TGRP):
                    ck = c0 + i
                    nc.tensor.matmul(
                        logT[:],
                        wout[:, ck, :],
                        ht[:, i, :],
                        start=(ck == 0),
                        stop=(ck == NCH - 1),
                        skip_group_check=True,
                    )

            grp = 0
            for tb in range(T // TBLK):
                xt = xt_pool.tile([K1, TBLK, 128], F16)
                nc.gpsimd.dma_start(
                    out=xt[:], in_=xt_d[:, bt, tb * TBLK:(tb + 1) * TBLK, :]
                )
                # tga = tanh of all gate cols: [tanh(i/2), tanh(o/2), tanh(g)]
                # (i, o pre-scaled by 0.5 in wmov on the host)
                tga = act_pool.tile([128, TBLK, NG], F16)
                uu = act_pool.tile([128, TBLK, D], F16)
                vv = act_pool.tile([128, TBLK, D], F16)
                tcc = act_pool.tile([128, TBLK, D], F16)
                ww = act_pool.tile([128, TBLK, D], F16)
                for j in range(TBLK):
                    t = tb * TBLK + j
                    s = t % GSLOTS
                    nc.tensor.matmul(
                        gates[:, s, 0:NG],
                        xt[:, j, :],
                        wmov[:],
                        start=True,
                        stop=True,
                        skip_group_check=True,
                    )
                    if s % ABATCH == ABATCH - 1:
                        j0, s0 = j - (ABATCH - 1), s - (ABATCH - 1)
                        nc.scalar.activation(
                            tga[:, j0:j + 1, :],
                            gates[:, s0:s + 1, 0:NG],
                            AF.Tanh,
                        )
                # c = sig(i)*tanh(g) = 0.5*(tg + ti2*tg); tc = tanh(c)
                # h' = 2h = tc + to2*tc  (the 0.5 folds into wout)
                ti2 = tga[:, :, 0:D]
                to2 = tga[:, :, D:2 * D]
                tgg = tga[:, :, 2 * D:NG]
                nc.vector.tensor_mul(uu[:], ti2, tgg)
                nc.vector.tensor_add(vv[:], tgg, uu[:])
                nc.scalar.activation(tcc[:], vv[:], AF.Tanh, scale=0.5)
                nc.vector.tensor_mul(ww[:], to2, tcc[:])
                hv = h[:, tb * TBLK * D:(tb + 1) * TBLK * D].rearrange(
                    "p (t d) -> p t d", d=D
                )
                nc.vector.tensor_add(hv, tcc[:], ww[:])
                # drain completed groups of TGRP td-chunks into mm2
                while (grp + 1) * TGRP * 128 <= (tb + 1) * TBLK * D:
                    mm2_group(grp)
                    grp += 1
            while grp < NCH // TGRP:  # tail group (includes the zero pad)
                mm2_group(grp)
                grp += 1
            logTs.append(logT)

        # softmax for both btiles at the end (one exp table load)
        for bt in range(NBT):
            logT = logTs[bt]
            lsb = fin_pool.tile([NCLS, 128], F32)
            nc.vector.tensor_add(lsb[:], logT[:], bout[:])
            smax = pl_pool.tile([128, NCLS], F32, tag="lg")
            nc.tensor.transpose(smax[:], lsb[:], id40[:])
            esb = fin_pool.tile([128, NCLS], F32)
            sums = fin_pool.tile([128, NGRP], F32)
            for g in range(NGRP):
                nc.scalar.activation(
                    esb[:, g * GRP:(g + 1) * GRP],
                    smax[:, g * GRP:(g + 1) * GRP],
                    AF.Exp,
                    accum_out=sums[:, g:g + 1],
                )
            rcp = fin_pool.tile([128, NGRP], F32)
            nc.vector.reciprocal(rcp[:], sums[:])
            ob = fin_pool.tile([128, NCLS], F32)
            for g in range(NGRP):
                nc.scalar.activation(
                    ob[:, g * GRP:(g + 1) * GRP],
                    esb[:, g * GRP:(g + 1) * GRP],
                    AF.Copy,
                    scale=rcp[:, g:g + 1],
                )
            nc.sync.dma_start(out=out_d[bt * 128:(bt + 1) * 128, :], in_=ob[:])

    nc.compile()
    return nc


def get_nc():
    if "nc" not in _CACHE:
        _CACHE["nc"] = _build_nc()
    return _CACHE["nc"]


def make_in_maps(x, W_ih, W_hh, b_ih, b_hh, W_out, b_out):
    f16 = mybir.dt.np(F16)
    bias = (np.asarray(b_ih, np.float32) + np.asarray(b_hh, np.float32))

    # moving operand [91, 270]: cols [i, o, g]; row 90 carries the bias.
    # i and o are pre-scaled by 0.5: sigmoid(x) = 0.5*(1 + tanh(x/2)) lets a
    # single tanh cover all gate columns (the 0.5 factors fold into wout).
    wmov = np.zeros((K1, NG), np.float32)
    wmov[0:D, 0:D] = 0.5 * np.asarray(W_ih)[0:D].T            # i
    wmov[0:D, D:2 * D] = 0.5 * np.asarray(W_ih)[3 * D:4 * D].T  # o
    wmov[0:D, 2 * D:NG] = np.asarray(W_ih)[2 * D:3 * D].T       # g
    wmov[D, 0:D] = 0.5 * bias[0:D]
    wmov[D, D:2 * D] = 0.5 * bias[3 * D:4 * D]
    wmov[D, 2 * D:NG] = bias[2 * D:3 * D]
    wmov = wmov.astype(f16)

    # device h' = tc + to2*tc = 2*sig(o)*tanh(c) = 2h -> scale wout by 0.5
    wt = np.zeros((TDP, NCLS), np.float32)
    wt[0:TD] = 0.5 * np.asarray(W_out, np.float32).T
    wout = np.ascontiguousarray(
        wt.reshape(NCH, 128, NCLS).transpose(1, 0, 2)
    ).astype(f16)

    boutr = np.ascontiguousarray(
        np.broadcast_to(np.asarray(b_out, np.float32)[:, None], (NCLS, 128))
    )
    id40 = np.eye(NCLS, dtype=np.float32)

    # x [2048, 240, 90] -> per core [91, btile, t, b] fp16 with ones row
    xs = np.asarray(x, np.float32).reshape(NCORES, NBT, 128, T, D)
    xt_all = np.empty((NCORES, K1, NBT, T, 128), f16)
    xt_all[:, 0:D] = xs.transpose(0, 4, 1, 3, 2).astype(f16)
    xt_all[:, D] = np.array(1.0, f16)

    return [
        {
            "xt": np.ascontiguousarray(xt_all[c]),
            "wmov": wmov,
            "woutT": wout,
            "bout": boutr,
            "id40": id40,
        }
        for c in range(NCORES)
    ]


def kernel(x, W_ih, W_hh, b_ih, b_hh, W_out, b_out, trace=False, **run_kwargs):
    nc = get_nc()
    in_maps = make_in_maps(x, W_ih, W_hh, b_ih, b_hh, W_out, b_out)
    res = run_bass_kernel_spmd(
        nc, in_maps, list(range(NCORES)), trace=trace, **run_kwargs
    )
    out = np.concatenate([res.results[c]["out"] for c in range(NCORES)], axis=0)
    out = out.reshape(B, NGRP, GRP).astype(np.float32)
    if trace:
        kernel.last_result = res
    return out


# revision 14
# speedup vs baseline: 1.9594x; 1.0166x over previous
"""Trainium2 Bass kernel for nn_Decoder (degenerate LSTM decoder).

Math (see reference):
  gates = x @ W_ih^T + (b_ih + b_hh)      [B, T, 4D], gate order i, f, g, o
  c = sigmoid(i) * tanh(g)                (f unused: c0 = 0)
  h = sigmoid(o) * tanh(c)                [B, T, D]
  out = softmax((h.reshape(B, T*D) @ W_out^T + b_out).reshape(B, 4, 10), axis=2)

Strategy: pure data parallel over 8 cores (batch 2048 -> 256/core).
Per core, batch stays on SBUF partitions everywhere:
  - host pre-transposes x to [d+1, btile, t, b] fp16 (ones row folds the bias
    into the K=91 contraction), so mm1 is lhsT=x_t^T [91,128], rhs=W [91,270]
    -> gates [128b, 270] in PSUM (col order i, o, g).
  - ACT applies sigmoid to [0:180] (i,o) and tanh to [180:270] (g) straight
    from PSUM in 3-timestep batches; DVE does the two fp16 multiplies.
  - h [128, 21632] fp16 is transposed in 128-wide td-chunks by the DMA xbar;
    mm2 accumulates logitsT [40, 128] over 169 chunks in PSUM.
  - logitsT + b_out, PE-transpose to [128, 40], exp with per-group accum sums,
    reciprocal, scale -> softmax; store fp32.
"""

import numpy as np
from contextlib import ExitStack

import concourse.bass as bass
import concourse.bacc as bacc
import concourse.tile as tile
from concourse import mybir
from concourse.bass_utils import run_bass_kernel_spmd

F16 = mybir.dt.float16
F32 = mybir.dt.float32
AF = mybir.ActivationFunctionType

B, T, D = 2048, 240, 90
NCLS, NGRP, GRP = 40, 4, 10
NCORES = 8
BC = B // NCORES            # 256 batch rows per core
NBT = 2                     # 128-row btiles per core
K1 = D + 1                  # 91 = d + ones row (bias)
NG = 3 * D                  # 270 gate cols: i(90) o(90) g(90)
TD = T * D                  # 21600
NCH = (TD + 127) // 128     # 169
TDP = NCH * 128             # 21632
TBLK = 12                   # t-block for DMA/elementwise staging
GSLOTS = 6                  # gates PSUM rotation depth (6 banks)
ABATCH = 3                  # timesteps per ACT instruction over gates PSUM
TGRP = 13                   # td-chunks per batched DMA transpose (13*13=169)

_CACHE: dict = {}


def _build_nc():
    nc = bacc.Bacc("TRN2", target_bir_lowering=False, debug=False)

    xt_d = nc.dram_tensor("xt", [K1, NBT, T, 128], F16, kind="ExternalInput")
    wmov_d = nc.dram_tensor("wmov", [K1, NG], F16, kind="ExternalInput")
    wout_d = nc.dram_tensor("woutT", [128, NCH, NCLS], F16, kind="ExternalInput")
    bout_d = nc.dram_tensor("bout", [NCLS, 128], F32, kind="ExternalInput")
    id40_d = nc.dram_tensor("id40", [NCLS, NCLS], F32, kind="ExternalInput")
    out_d = nc.dram_tensor("out", [BC, NCLS], F32, kind="ExternalOutput")

    with ExitStack() as ctx:
        tc = ctx.enter_context(tile.TileContext(nc))
        consts = ctx.enter_context(tc.tile_pool(name="consts", bufs=1))
        xt_pool = ctx.enter_context(tc.tile_pool(name="xt", bufs=4))
        act_pool = ctx.enter_context(tc.tile_pool(name="acts", bufs=4))
        h_pool = ctx.enter_context(tc.tile_pool(name="h", bufs=2))
        ht_pool = ctx.enter_context(tc.tile_pool(name="ht", bufs=3))
        fin_pool = ctx.enter_context(tc.tile_pool(name="fin", bufs=2))
        pg_pool = ctx.enter_context(tc.tile_pool(name="pg", bufs=1, space="PSUM"))
        pl_pool = ctx.enter_context(tc.tile_pool(name="pl", bufs=2, space="PSUM"))

        wmov = consts.tile([K1, NG], F16)
        nc.gpsimd.dma_start(out=wmov[:], in_=wmov_d[:])
        wout = consts.tile([128, NCH, NCLS], F16)
        nc.gpsimd.dma_start(out=wout[:], in_=wout_d[:])
        bout = consts.tile([NCLS, 128], F32)
        nc.gpsimd.dma_start(out=bout[:], in_=bout_d[:])
        id40 = consts.tile([NCLS, NCLS], F32)
        nc.gpsimd.dma_start(out=id40[:], in_=id40_d[:])

        logTs = []
        for bt in range(NBT):
            h = h_pool.tile([128, TDP], F16)
            # zero the tail so the padded td-chunk contributes nothing
            nc.vector.memset(h[:, TD:TDP], 0.0)
            gates = pg_pool.tile([128, GSLOTS, 512], F32)
            logT = pl_pool.tile([NCLS, 128], F32, tag="lg")

            pending = []

            def transpose_group(g):
                # batched xbar transpose of TGRP td-chunks
                ht = ht_pool.tile([128, TGRP, 128], F16)
                c0 = g * TGRP
                nc.sync.dma_start(
                    out=ht[:],
                    in_=h[:, c0 * 128:(c0 + TGRP) * 128],
                    transpose=True,
                )
                pending.append((g, ht))

            def mm2_flush(keep):
                # emit mm2 matmuls lagging the transposes so they never
                # stall at the head of the PE queue
                while len(pending) > keep:
                    g, ht = pending.pop(0)
                    for i in range(TGRP):
                        ck = g * TGRP + i
                        nc.tensor.matmul(
                            logT[:],
                            wout[:, ck, :],
                            ht[:, i, :],
                            start=(ck == 0),
                            stop=(ck == NCH - 1),
                            skip_group_check=True,
                        )

            grp = 0
            for tb in range(T // TBLK):
                xt = xt_pool.tile([K1, TBLK, 128], F16)
                nc.gpsimd.dma_start(
                    out=xt[:], in_=xt_d[:, bt, tb * TBLK:(tb + 1) * TBLK, :]
                )
                # tga = tanh of all gate cols: [tanh(i/2), tanh(o/2), tanh(g)]
                # (i, o pre-scaled by 0.5 in wmov on the host)
                tga = act_pool.tile([128, TBLK, NG], F16)
                uu = act_pool.tile([128, TBLK, D], F16)
                vv = act_pool.tile([128, TBLK, D], F16)
                tcc = act_pool.tile([128, TBLK, D], F16)
                ww = act_pool.tile([128, TBLK, D], F16)
                for j in range(TBLK):
                    t = tb * TBLK + j
                    s = t % GSLOTS
                    nc.tensor.matmul(
                        gates[:, s, 0:NG],
                        xt[:, j, :],
                        wmov[:],
                        start=True,
                        stop=True,
                        skip_group_check=True,
                    )
                    if s % ABATCH == ABATCH - 1:
                        j0, s0 = j - (ABATCH - 1), s - (ABATCH - 1)
                        nc.scalar.activation(
                            tga[:, j0:j + 1, :],
                            gates[:, s0:s + 1, 0:NG],
                            AF.Tanh,
                        )
                # c = sig(i)*tanh(g) = 0.5*tg*(1+ti2); tc = tanh(c)
                # h' = 2h = tc*(1+to2)  (the 0.5 folds into wout)
                ti2 = tga[:, :, 0:D]
                to2 = tga[:, :, D:2 * D]
                tgg = tga[:, :, 2 * D:NG]
                nc.vector.tensor_scalar_add(uu[:], ti2, 1.0)  # 4x-rate
                nc.vector.tensor_mul(vv[:], tgg, uu[:])
                nc.scalar.activation(tcc[:], vv[:], AF.Tanh, scale=0.5)
                nc.vector.tensor_scalar_add(ww[:], to2, 1.0)  # 4x-rate
                hv = h[:, tb * TBLK * D:(tb + 1) * TBLK * D].rearrange(
                    "p (t d) -> p t d", d=D
                )
                nc.vector.tensor_mul(hv, tcc[:], ww[:])
                # drain completed groups of TGRP td-chunks into mm2
                while (grp + 1) * TGRP * 128 <= (tb + 1) * TBLK * D:
                    transpose_group(grp)
                    grp += 1
                    mm2_flush(keep=1)
            while grp < NCH // TGRP:  # tail group (includes the zero pad)
                transpose_group(grp)
                grp += 1
            mm2_flush(keep=0)
            logTs.append(logT)

        # softmax for both btiles at the end (one exp table load)
        for bt in range(NBT):
            logT = logTs[bt]
            lsb = fin_pool.tile([NCLS, 128], F32)
            nc.vector.tensor_add(lsb[:], logT[:], bout[:])
            smax = pl_pool.tile([128, NCLS], F32, tag="lg")
            nc.tensor.transpose(smax[:], lsb[:], id40[:])
            esb = fin_pool.tile([128, NCLS], F32)
            sums = fin_pool.tile([128, NGRP], F32)
            for g in range(NGRP):
                nc.scalar.activation(
                    esb[:, g * GRP:(g + 1) * GRP],
                    smax[:, g * GRP:(g + 1) * GRP],
                    AF.Exp,
                    accum_out=sums[:, g:g + 1],
                )
            rcp = fin_pool.tile([128, NGRP], F32)
            nc.vector.reciprocal(rcp[:], sums[:])
            ob = fin_pool.tile([128, NCLS], F32)
            for g in range(NGRP):
                nc.scalar.activation(
                    ob[:, g * GRP:(g + 1) * GRP],
                    esb[:, g * GRP:(g + 1) * GRP],
                    AF.Copy,
                    scale=rcp[:, g:g + 1],
                )
            nc.sync.dma_start(out=out_d[bt * 128:(bt + 1) * 128, :], in_=ob[:])

    nc.compile()
    return nc


def get_nc():
    if "nc" not in _CACHE:
        _CACHE["nc"] = _build_nc()
    return _CACHE["nc"]


def make_in_maps(x, W_ih, W_hh, b_ih, b_hh, W_out, b_out):
    f16 = mybir.dt.np(F16)
    bias = (np.asarray(b_ih, np.float32) + np.asarray(b_hh, np.float32))

    # moving operand [91, 270]: cols [i, o, g]; row 90 carries the bias.
    # i and o are pre-scaled by 0.5: sigmoid(x) = 0.5*(1 + tanh(x/2)) lets a
    # single tanh cover all gate columns (the 0.5 factors fold into wout).
    wmov = np.zeros((K1, NG), np.float32)
    wmov[0:D, 0:D] = 0.5 * np.asarray(W_ih)[0:D].T            # i
    wmov[0:D, D:2 * D] = 0.5 * np.asarray(W_ih)[3 * D:4 * D].T  # o
    wmov[0:D, 2 * D:NG] = np.asarray(W_ih)[2 * D:3 * D].T       # g
    wmov[D, 0:D] = 0.5 * bias[0:D]
    wmov[D, D:2 * D] = 0.5 * bias[3 * D:4 * D]
    wmov[D, 2 * D:NG] = bias[2 * D:3 * D]
    wmov = wmov.astype(f16)

    # device h' = tc + to2*tc = 2*sig(o)*tanh(c) = 2h -> scale wout by 0.5
    wt = np.zeros((TDP, NCLS), np.float32)
    wt[0:TD] = 0.5 * np.asarray(W_out, np.float32).T
    wout = np.ascontiguousarray(
        wt.reshape(NCH, 128, NCLS).transpose(1, 0, 2)
    ).astype(f16)

    boutr = np.ascontiguousarray(
        np.broadcast_to(np.asarray(b_out, np.float32)[:, None], (NCLS, 128))
    )
    id40 = np.eye(NCLS, dtype=np.float32)

    # x [2048, 240, 90] -> per core [91, btile, t, b] fp16 with ones row
    xs = np.asarray(x, np.float32).reshape(NCORES, NBT, 128, T, D)
    xt_all = np.empty((NCORES, K1, NBT, T, 128), f16)
    xt_all[:, 0:D] = xs.transpose(0, 4, 1, 3, 2).astype(f16)
    xt_all[:, D] = np.array(1.0, f16)

    return [
        {
            "xt": np.ascontiguousarray(xt_all[c]),
            "wmov": wmov,
            "woutT": wout,
            "bout": boutr,
            "id40": id40,
        }
        for c in range(NCORES)
    ]


def kernel(x, W_ih, W_hh, b_ih, b_hh, W_out, b_out, trace=False, **run_kwargs):
    nc = get_nc()
    in_maps = make_in_maps(x, W_ih, W_hh, b_ih, b_hh, W_out, b_out)
    res = run_bass_kernel_spmd(
        nc, in_maps, list(range(NCORES)), trace=trace, **run_kwargs
    )
    out = np.concatenate([res.results[c]["out"] for c in range(NCORES)], axis=0)
    out = out.reshape(B, NGRP, GRP).astype(np.float32)
    if trace:
        kernel.last_result = res
    return out


# revision 19
# speedup vs baseline: 2.5860x; 1.3198x over previous
"""Trainium2 Bass kernel for nn_Decoder (degenerate LSTM decoder).

Math (see reference):
  gates = x @ W_ih^T + (b_ih + b_hh)      [B, T, 4D], gate order i, f, g, o
  c = sigmoid(i) * tanh(g)                (f unused: c0 = 0)
  h = sigmoid(o) * tanh(c)                [B, T, D]
  out = softmax((h.reshape(B, T*D) @ W_out^T + b_out).reshape(B, 4, 10), axis=2)

Strategy: pure data parallel over 8 cores (batch 2048 -> 256/core).
Per core, batch stays on SBUF partitions everywhere:
  - host pre-transposes x to [d+1, btile, t, b] fp16 (ones row folds the bias
    into the K=91 contraction), so mm1 is lhsT=x_t^T [91,128], rhs=W [91,270]
    -> gates [128b, 270] in PSUM (col order i, o, g).
  - ACT applies sigmoid to [0:180] (i,o) and tanh to [180:270] (g) straight
    from PSUM in 3-timestep batches; DVE does the two fp16 multiplies.
  - h [128, 21632] fp16 is transposed in 128-wide td-chunks by the DMA xbar;
    mm2 accumulates logitsT [40, 128] over 169 chunks in PSUM.
  - logitsT + b_out, PE-transpose to [128, 40], exp with per-group accum sums,
    reciprocal, scale -> softmax; store fp32.
"""

import numpy as np
from contextlib import ExitStack

import concourse.bass as bass
import concourse.bacc as bacc
import concourse.tile as tile
from concourse import mybir
from concourse.bass_utils import run_bass_kernel_spmd

F16 = mybir.dt.float16
F32 = mybir.dt.float32
AF = mybir.ActivationFunctionType

B, T, D = 2048, 240, 90
NCLS, NGRP, GRP = 40, 4, 10
NCORES = 8
BC = B // NCORES            # 256 batch rows per core
NBT = 2                     # 128-row btiles per core
K1 = D + 1                  # 91 = d + ones row (bias)
NG = 3 * D                  # 270 gate cols: i(90) o(90) g(90)
TD = T * D                  # 21600
NCH = (TD + 127) // 128     # 169
TDP = NCH * 128             # 21632
TBLK = 12                   # t-block for DMA/elementwise staging
GSLOTS = 3                  # gates PSUM rotation depth per btile (3 banks x 2)
ABATCH = 3                  # timesteps per ACT instruction over gates PSUM
TGRP = 13                   # td-chunks per batched DMA transpose (13*13=169)

_CACHE: dict = {}


def _build_nc():
    nc = bacc.Bacc("TRN2", target_bir_lowering=False, debug=False)

    xt_d = nc.dram_tensor("xt", [K1, NBT, T, 128], F16, kind="ExternalInput")
    wmov_d = nc.dram_tensor("wmov", [K1, NG], F16, kind="ExternalInput")
    wout_d = nc.dram_tensor("woutT", [128, NCH, NCLS], F16, kind="ExternalInput")
    bout_d = nc.dram_tensor("bout", [NCLS, 128], F32, kind="ExternalInput")
    id40_d = nc.dram_tensor("id40", [NCLS, NCLS], F32, kind="ExternalInput")
    out_d = nc.dram_tensor("out", [BC, NCLS], F32, kind="ExternalOutput")

    with ExitStack() as ctx:
        tc = ctx.enter_context(tile.TileContext(nc))
        consts = ctx.enter_context(tc.tile_pool(name="consts", bufs=1))
        xt_pool = ctx.enter_context(tc.tile_pool(name="xt", bufs=4))
        act_pool = ctx.enter_context(tc.tile_pool(name="acts", bufs=6))
        h_pool = ctx.enter_context(tc.tile_pool(name="h", bufs=2))
        ht_pool = ctx.enter_context(tc.tile_pool(name="ht", bufs=4))
        fin_pool = ctx.enter_context(tc.tile_pool(name="fin", bufs=2))
        pg_pool = ctx.enter_context(tc.tile_pool(name="pg", bufs=2, space="PSUM"))
        pl_pool = ctx.enter_context(tc.tile_pool(name="pl", bufs=2, space="PSUM"))

        wmov = consts.tile([K1, NG], F16)
        nc.gpsimd.dma_start(out=wmov[:], in_=wmov_d[:])
        wout = consts.tile([128, NCH, NCLS], F16)
        nc.gpsimd.dma_start(out=wout[:], in_=wout_d[:])
        bout = consts.tile([NCLS, 128], F32)
        nc.gpsimd.dma_start(out=bout[:], in_=bout_d[:])
        id40 = consts.tile([NCLS, NCLS], F32)
        nc.gpsimd.dma_start(out=id40[:], in_=id40_d[:])

        # Both btiles are interleaved in the t-loop: two independent
        # mm1 <-> tanh PSUM dependency chains keep PE and ACT saturated.
        hs, logTs, gates_bt = [], [], []
        for bt in range(NBT):
            h = h_pool.tile([128, TDP], F16, tag="h")
            # zero the tail so the padded td-chunk contributes nothing
            nc.vector.memset(h[:, TD:TDP], 0.0)
            hs.append(h)
            logTs.append(pl_pool.tile([NCLS, 128], F32, tag="lg", name=f"logT{bt}"))
            gates_bt.append(pg_pool.tile([128, GSLOTS, 512], F32, tag="g", name=f"gates{bt}"))

        mm2_pend = [[], []]
        grp = [0, 0]

        def transpose_group(bt):
            g = grp[bt]
            ht = ht_pool.tile([128, TGRP, 128], F16, tag="ht")
            c0 = g * TGRP
            nc.sync.dma_start(
                out=ht[:],
                in_=hs[bt][:, c0 * 128:(c0 + TGRP) * 128],
                transpose=True,
            )
            mm2_pend[bt].append((g, ht))
            grp[bt] = g + 1

        def mm2_flush(bt, keep):
            # emit mm2 matmuls lagging the transposes so they never
            # stall at the head of the PE queue
            while len(mm2_pend[bt]) > keep:
                g, ht = mm2_pend[bt].pop(0)
                for i in range(TGRP):
                    ck = g * TGRP + i
                    nc.tensor.matmul(
                        logTs[bt][:],
                        wout[:, ck, :],
                        ht[:, i, :],
                        start=(ck == 0),
                        stop=(ck == NCH - 1),
                        skip_group_check=True,
                    )

        post = []  # (bt, tb, vv) awaiting tanh(c) — lagged one block

        def post_flush(keep):
            while len(post) > keep:
                bt, tb, vv = post.pop(0)
                tcc = act_pool.tile([128, TBLK, D], F16, tag="tcc", bufs=3)
                ww = act_pool.tile([128, TBLK, D], F16, tag="ww", bufs=3)
                nc.scalar.activation(tcc[:], vv[:], AF.Tanh, scale=0.5)
                nc.vector.tensor_scalar_add(ww[:], tga_slices[(bt, tb)], 1.0)
                hv = hs[bt][:, tb * TBLK * D:(tb + 1) * TBLK * D].rearrange(
                    "p (t d) -> p t d", d=D
                )
                nc.vector.tensor_mul(hv, tcc[:], ww[:])
                # drain completed groups of TGRP td-chunks into mm2
                while (grp[bt] + 1) * TGRP * 128 <= (tb + 1) * TBLK * D:
                    transpose_group(bt)
                    mm2_flush(bt, keep=1)

        tga_slices = {}
        for tb in range(T // TBLK):
            xts = []
            for bt in range(NBT):
                xt = xt_pool.tile([K1, TBLK, 128], F16, tag="xt")
                nc.gpsimd.dma_start(
                    out=xt[:], in_=xt_d[:, bt, tb * TBLK:(tb + 1) * TBLK, :]
                )
                xts.append(xt)
            # tga = tanh of all gate cols: [tanh(i/2), tanh(o/2), tanh(g)]
            # (i, o pre-scaled by 0.5 in wmov on the host)
            tgas = [
                act_pool.tile([128, TBLK, NG], F16, tag="tga", name=f"tga{bt}", bufs=5)
                for bt in range(NBT)
            ]
            for j in range(TBLK):
                t = tb * TBLK + j
                s = t % GSLOTS
                for bt in range(NBT):
                    nc.tensor.matmul(
                        gates_bt[bt][:, s, 0:NG],
                        xts[bt][:, j, :],
                        wmov[:],
                        start=True,
                        stop=True,
                        skip_group_check=True,
                    )
                if s == GSLOTS - 1:
                    j0, s0 = j - (GSLOTS - 1), 0
                    for bt in range(NBT):
                        nc.scalar.activation(
                            tgas[bt][:, j0:j + 1, :],
                            gates_bt[bt][:, s0:s + 1, 0:NG],
                            AF.Tanh,
                        )
            # c = sig(i)*tanh(g) = 0.5*tg*(1+ti2); tc = tanh(c)
            # h' = 2h = tc*(1+to2)  (the 0.5 folds into wout)
            for bt in range(NBT):
                tga = tgas[bt]
                ti2 = tga[:, :, 0:D]
                tgg = tga[:, :, 2 * D:NG]
                uu = act_pool.tile([128, TBLK, D], F16, tag="uu", bufs=3)
                vv = act_pool.tile([128, TBLK, D], F16, tag="vv", bufs=5)
                nc.vector.tensor_scalar_add(uu[:], ti2, 1.0)  # 4x-rate
                nc.vector.tensor_mul(vv[:], tgg, uu[:])
                tga_slices[(bt, tb)] = tga[:, :, D:2 * D]
                post.append((bt, tb, vv))
            post_flush(keep=NBT)
        post_flush(keep=0)
        for bt in range(NBT):
            while grp[bt] < NCH // TGRP:  # tail group (includes the zero pad)
                transpose_group(bt)
            mm2_flush(bt, keep=0)

        # softmax for both btiles at the end (one exp table load)
        for bt in range(NBT):
            logT = logTs[bt]
            lsb = fin_pool.tile([NCLS, 128], F32)
            nc.vector.tensor_add(lsb[:], logT[:], bout[:])
            smax = pl_pool.tile([128, NCLS], F32, tag="lg")
            nc.tensor.transpose(smax[:], lsb[:], id40[:])
            esb = fin_pool.tile([128, NCLS], F32)
            sums = fin_pool.tile([128, NGRP], F32)
            for g in range(NGRP):
                nc.scalar.activation(
                    esb[:, g * GRP:(g + 1) * GRP],
                    smax[:, g * GRP:(g + 1) * GRP],
                    AF.Exp,
                    accum_out=sums[:, g:g + 1],
                )
            rcp = fin_pool.tile([128, NGRP], F32)
            nc.vector.reciprocal(rcp[:], sums[:])
            ob = fin_pool.tile([128, NCLS], F32)
            for g in range(NGRP):
                nc.scalar.activation(
                    ob[:, g * GRP:(g + 1) * GRP],
                    esb[:, g * GRP:(g + 1) * GRP],
                    AF.Copy,
                    scale=rcp[:, g:g + 1],
                )
            nc.sync.dma_start(out=out_d[bt * 128:(bt + 1) * 128, :], in_=ob[:])

    nc.compile()
    return nc


def get_nc():
    if "nc" not in _CACHE:
        _CACHE["nc"] = _build_nc()
    return _CACHE["nc"]


def make_in_maps(x, W_ih, W_hh, b_ih, b_hh, W_out, b_out):
    f16 = mybir.dt.np(F16)
    bias = (np.asarray(b_ih, np.float32) + np.asarray(b_hh, np.float32))

    # moving operand [91, 270]: cols [i, o, g]; row 90 carries the bias.
    # i and o are pre-scaled by 0.5: sigmoid(x) = 0.5*(1 + tanh(x/2)) lets a
    # single tanh cover all gate columns (the 0.5 factors fold into wout).
    wmov = np.zeros((K1, NG), np.float32)
    wmov[0:D, 0:D] = 0.5 * np.asarray(W_ih)[0:D].T            # i
    wmov[0:D, D:2 * D] = 0.5 * np.asarray(W_ih)[3 * D:4 * D].T  # o
    wmov[0:D, 2 * D:NG] = np.asarray(W_ih)[2 * D:3 * D].T       # g
    wmov[D, 0:D] = 0.5 * bias[0:D]
    wmov[D, D:2 * D] = 0.5 * bias[3 * D:4 * D]
    wmov[D, 2 * D:NG] = bias[2 * D:3 * D]
    wmov = wmov.astype(f16)

    # device h' = tc + to2*tc = 2*sig(o)*tanh(c) = 2h -> scale wout by 0.5
    wt = np.zeros((TDP, NCLS), np.float32)
    wt[0:TD] = 0.5 * np.asarray(W_out, np.float32).T
    wout = np.ascontiguousarray(
        wt.reshape(NCH, 128, NCLS).transpose(1, 0, 2)
    ).astype(f16)

    boutr = np.ascontiguousarray(
        np.broadcast_to(np.asarray(b_out, np.float32)[:, None], (NCLS, 128))
    )
    id40 = np.eye(NCLS, dtype=np.float32)

    # x [2048, 240, 90] -> per core [91, btile, t, b] fp16 with ones row
    xs = np.asarray(x, np.float32).reshape(NCORES, NBT, 128, T, D)
    xt_all = np.empty((NCORES, K1, NBT, T, 128), f16)
    xt_all[:, 0:D] = xs.transpose(0, 4, 1, 3, 2).astype(f16)
    xt_all[:, D] = np.array(1.0, f16)

    return [
        {
            "xt": np.ascontiguousarray(xt_all[c]),
            "wmov": wmov,
            "woutT": wout,
            "bout": boutr,
            "id40": id40,
        }
        for c in range(NCORES)
    ]


def kernel(x, W_ih, W_hh, b_ih, b_hh, W_out, b_out, trace=False, **run_kwargs):
    nc = get_nc()
    in_maps = make_in_maps(x, W_ih, W_hh, b_ih, b_hh, W_out, b_out)
    res = run_bass_kernel_spmd(
        nc, in_maps, list(range(NCORES)), trace=trace, **run_kwargs
    )
    out = np.concatenate([res.results[c]["out"] for c in range(NCORES)], axis=0)
    out = out.reshape(B, NGRP, GRP).astype(np.float32)
    if trace:
        kernel.last_result = res
    return out
